# revision 16
# baseline (speedup 1.0000x reference)
"""Trainium2 Bass kernel for nn_MultiHeadDaubechiesBlock (v2).

Data-parallel over batch B=8 across 8 NeuronCores (one sequence per core).
Per-core pipeline:
  LN1 (DVE bn_stats, bf16 x resident in SBUF) -> DWT as Toeplitz-block
  matmuls (token-major, lvl1 interleaved with LN1) -> merged linear-interp
  upsample matmuls (feature-major out) -> proj GEMM bf16 + residual
  (rank-1 bias terms folded into the uploaded x on the host) -> LN2 ->
  FFN1 + FFN2 as fp8e4 DoubleRow GEMMs (2x PE rate), exact gelu on ACT.
Software-pipelined chunk loop: FFN2(c) is deferred past interp/proj(c+1)
so the gelu stream always drains before FFN2 consumes it and PE never
stalls; no warmup pacer (HAM warms during the DWT burst).
"""
import numpy as np
import ml_dtypes

B, T, D, H, DH, LEVELS, FFN = 8, 4096, 512, 4, 128, 3, 2048
P = 128
NT = T // P          # 32 token tiles
NDT = D // P         # 4 feature tiles
NFT = FFN // P       # 16 ffn tiles
NCH = 8              # t-chunks of 512
NWS = [2047, 1023, 511]
LPADS = [4096, 2048, 1024]
EPS = 1e-5
BF16 = ml_dtypes.bfloat16
FP8 = ml_dtypes.float8_e4m3


# ----------------------------------------------------------------- plan
def _interp_mat(L, out_size=T):
    src = np.maximum((np.arange(out_size, dtype=np.float64) + 0.5) * (L / out_size) - 0.5, 0.0)
    i0 = np.clip(np.floor(src).astype(np.int64), 0, L - 1)
    i1 = np.minimum(i0 + 1, L - 1)
    w = src - i0
    U = np.zeros((out_size, L), np.float64)
    U[np.arange(out_size), i0] += 1.0 - w
    U[np.arange(out_size), i1] += w
    return U.astype(np.float32)


def make_plan():
    """Input-value-independent schedule + interp weight blocks."""
    dwt = []
    for lvl in range(LEVELS):
        nw = NWS[lvl]
        ng = (nw + 63) // 64
        dwt.append([(g, g == ng - 1) for g in range(ng)])

    Ls = [NWS[0], NWS[1], NWS[2], NWS[2]]
    Us = [_interp_mat(L) for L in Ls]
    ublks = []
    isched = [[] for _ in range(NCH)]
    for c in range(NCH):
        for s in range(4):
            U, L = Us[s], Ls[s]
            cols = U[512 * c:512 * (c + 1)]           # [512, L]
            nz = np.nonzero(cols.any(0))[0]
            for kt in range(nz.min() // P, nz.max() // P + 1):
                K = min(P, L - P * kt)
                blk = cols[:, P * kt:P * kt + K].T    # [K, 512]
                if not np.any(blk):
                    continue
                full = np.zeros((P, 512), np.float32)
                full[:K] = blk
                isched[c].append((s, kt, K, len(ublks)))
                ublks.append(full)
    ublk = np.stack(ublks)                            # [NB, 128, 512] f32
    return {"dwt": dwt, "isched": isched, "ublk": ublk, "nb": len(ublks)}


def _toeplitz(nw, Lp, f):
    F = np.zeros((nw, Lp), np.float32)
    for w in range(nw):
        F[w, 2 * w:2 * w + 4] = f
    return F


def make_consts(inputs, plan):
    """Host-side constants (depend on input values)."""
    h0, h1 = np.asarray(inputs["h0"]), np.asarray(inputs["h1"])
    f0 = h0[:, 0, :, 0].astype(np.float32)
    f1 = h1[:, 0, :, 0].astype(np.float32)
    ln1_g = np.asarray(inputs["ln1_g"], np.float32)
    ln2_g = np.asarray(inputs["ln2_g"], np.float32)
    ln2_b = np.asarray(inputs["ln2_b"], np.float32)
    proj_w = np.asarray(inputs["proj_w"], np.float32)
    w1 = np.asarray(inputs["w1"], np.float32)
    b1 = np.asarray(inputs["b1"], np.float32)
    w2 = np.asarray(inputs["w2"], np.float32)
    b2 = np.asarray(inputs["b2"], np.float32)

    # merged DWT lhsT blocks [9,128,128]: cols 0..63 low (f0), 64..127 high (f1)
    fblk = np.zeros((9, P, P), np.float32)
    for lvl in range(LEVELS):
        A = fblk[lvl * 3 + 0]
        for r in range(P):
            for w in range(64):
                k = r - 2 * w
                if 0 <= k < 4:
                    A[r, w] = f0[lvl][k]
                    A[r, 64 + w] = f1[lvl][k]
        Bt = fblk[lvl * 3 + 1]
        for r in range(2):
            Bt[r, 63] = f0[lvl][r + 2]
            Bt[r, 127] = f1[lvl][r + 2]
        Al = fblk[lvl * 3 + 2]
        Al[:] = A
        Al[:, 63] = 0.0
        Al[:, 127] = 0.0

    wg = (ln1_g[:, None] * proj_w)                # LN1 g fold
    w1g = (ln2_g[:, None] * w1)                   # LN2 g fold
    b1f = b1 + ln2_b @ w1                         # LN2 b fold

    return {
        "wg": wg.astype(BF16),
        "w1": w1g.astype(FP8),
        "w2": w2.astype(FP8),
        "fblk": fblk.reshape(9 * P, P).astype(BF16),          # [9*128, 128]
        "ublk": plan["ublk"].reshape(-1, 512).astype(BF16),   # [NB*128, 512]
        "b1c": np.ascontiguousarray(b1f.reshape(NFT, P).T.astype(np.float32)),  # [128,16]
        "b2r": b2.reshape(1, D).astype(BF16),                 # [1, 512]
        "idn": np.identity(P, np.float32).astype(BF16),       # [128,128]
    }


def make_xfold(inputs):
    """xr = x + 1*proj_b + m1*(ln1_b@proj_w), the rank-1 proj bias terms."""
    f0 = np.asarray(inputs["h0"])[:, 0, :, 0].astype(np.float32)
    f1 = np.asarray(inputs["h1"])[:, 0, :, 0].astype(np.float32)
    proj_b = np.asarray(inputs["proj_b"], np.float32)
    ln1_b = np.asarray(inputs["ln1_b"], np.float32)
    proj_w = np.asarray(inputs["proj_w"], np.float32)
    bW = ln1_b @ proj_w
    ones = np.ones((T, 1), np.float32)
    a = ones
    comb = np.zeros((T, 1), np.float32)
    Us = [_interp_mat(L) for L in [NWS[0], NWS[1], NWS[2], NWS[2]]]
    for lvl in range(LEVELS):
        ap_ = np.zeros((LPADS[lvl], 1), np.float32)
        ap_[:a.shape[0]] = a
        comb += Us[lvl] @ (_toeplitz(NWS[lvl], LPADS[lvl], f1[lvl]) @ ap_)
        a = _toeplitz(NWS[lvl], LPADS[lvl], f0[lvl]) @ ap_
    comb += Us[3] @ a
    m1 = comb[:, 0]                               # [T]
    x = np.asarray(inputs["x"], np.float32)
    xr = x + proj_b[None, None, :] + m1[None, :, None] * bW[None, None, :]
    return xr.astype(BF16)


# ----------------------------------------------------------------- bass
def build_nc(plan):
    import concourse.bacc as bacc
    import concourse.tile as tile
    from concourse import mybir

    F32, BF, F8 = mybir.dt.float32, mybir.dt.bfloat16, mybir.dt.float8e4
    AF = mybir.ActivationFunctionType
    OP = mybir.AluOpType
    DR = mybir.MatmulPerfMode.DoubleRow

    nc = bacc.Bacc("TRN2", target_bir_lowering=False, debug=False, name="daub2")
    x_d = nc.dram_tensor("x", [T, D], BF, kind="ExternalInput")
    out_d = nc.dram_tensor("out", [T, D], F32, kind="ExternalOutput")
    wg_d = nc.dram_tensor("wg", [D, D], BF, kind="ExternalInput")
    w1_d = nc.dram_tensor("w1", [D, FFN], F8, kind="ExternalInput")
    w2_d = nc.dram_tensor("w2", [FFN, D], F8, kind="ExternalInput")
    fblk_d = nc.dram_tensor("fblk", [9 * P, P], BF, kind="ExternalInput")
    ublk_d = nc.dram_tensor("ublk", [plan["nb"] * P, 512], BF, kind="ExternalInput")
    b1c_d = nc.dram_tensor("b1c", [P, NFT], F32, kind="ExternalInput")
    b2r_d = nc.dram_tensor("b2r", [1, D], BF, kind="ExternalInput")
    idn_d = nc.dram_tensor("idn", [P, P], BF, kind="ExternalInput")

    with tile.TileContext(nc) as tc:
        import contextlib
        ctx = contextlib.ExitStack()
        pw = ctx.enter_context(tc.tile_pool(name="pw", bufs=1))
        pbig = ctx.enter_context(tc.tile_pool(name="pbig", bufs=1))
        pio = ctx.enter_context(tc.tile_pool(name="pio", bufs=4))
        pu = ctx.enter_context(tc.tile_pool(name="pu", bufs=24))
        px2 = ctx.enter_context(tc.tile_pool(name="px2", bufs=8))
        pcomb = ctx.enter_context(tc.tile_pool(name="pcomb", bufs=2))
        pxn2 = ctx.enter_context(tc.tile_pool(name="pxn2", bufs=2))
        ptm = ctx.enter_context(tc.tile_pool(name="ptm", bufs=8))
        psA = ctx.enter_context(tc.tile_pool(name="psA", bufs=2, space="PSUM"))
        psB = ctx.enter_context(tc.tile_pool(name="psB", bufs=2, space="PSUM"))
        psF = ctx.enter_context(tc.tile_pool(name="psF", bufs=2, space="PSUM"))
        psO = ctx.enter_context(tc.tile_pool(name="psO", bufs=2, space="PSUM"))

        # ---- small consts
        idn_sb = pw.tile([P, P], BF, name="idn_sb")
        nc.sync.dma_start(out=idn_sb, in_=idn_d[:, :])
        fblk_sb = pw.tile([P, 9, P], BF, name="fblk_sb")
        nc.sync.dma_start(out=fblk_sb, in_=fblk_d.rearrange("(b p) m -> p b m", p=P))
        b1c_sb = pw.tile([P, NFT], F32, name="b1c_sb")
        nc.sync.dma_start(out=b1c_sb, in_=b1c_d[:, :])
        b2r_sb = pw.tile([1, D], BF, name="b2r_sb")
        nc.sync.dma_start(out=b2r_sb, in_=b2r_d[:, :])
        eps_sb = pw.tile([P, 1], F32, name="eps_sb")
        nc.vector.memset(eps_sb, EPS)
        ones_sb = pw.tile([1, P], BF, name="ones_sb")
        nc.vector.memset(ones_sb, 1.0)

        # ---- big activations
        xr_sb = pbig.tile([P, NT, D], BF, name="xr_sb")      # resident x (folded)
        xn_sb = pbig.tile([P, 4, D], BF, name="xn_sb")       # LN1 out ring
        a1 = pbig.tile([P, 16, D], BF, name="a1")
        a2 = pbig.tile([P, 8, D], BF, name="a2")
        a3 = pbig.tile([P, 4, D], BF, name="a3")
        d0 = pbig.tile([P, 16, D], BF, name="d0")
        hdn = pbig.tile([P, NT, D], F8, name="hdn")
        wg_sb = pw.tile([P, NDT, D], BF, name="wg_sb")
        w1_sb = pw.tile([P, NDT, FFN], F8, name="w1_sb")
        w2_sb = pw.tile([P, NFT, D], F8, name="w2_sb")

        # zero the cascade-input pad rows (evacs later overwrite 96..126;
        # partition base must be 32-aligned so a 1-row memset is illegal)
        nc.vector.memset(a1[96:128, 15, :], 0.0)
        nc.vector.memset(a2[96:128, 7, :], 0.0)

        # ---- x upload (8 chunky DMAs), then weights, then ub c0/c1
        xrr = x_d.rearrange("(t p) d -> p t d", p=P)
        for j in range(8):
            nc.sync.dma_start(out=xr_sb[:, 4 * j:4 * j + 4, :], in_=xrr[:, 4 * j:4 * j + 4, :])
        nc.sync.dma_start(out=wg_sb, in_=wg_d.rearrange("(kt p) n -> p kt n", p=P))
        nc.sync.dma_start(out=w1_sb, in_=w1_d.rearrange("(kt p) n -> p kt n", p=P))
        nc.sync.dma_start(out=w2_sb, in_=w2_d.rearrange("(kt p) n -> p kt n", p=P))

        ubh = {}

        def emit_ub_dma(c, eng):
            for (s, kt, K, idx) in plan["isched"][c]:
                ut = pu.tile([P, 512], BF, tag="ub", name=f"ub{idx}")
                eng.dma_start(out=ut, in_=ublk_d[P * idx:P * (idx + 1), :])
                ubh[idx] = ut

        emit_ub_dma(0, nc.sync)
        emit_ub_dma(1, nc.sync)

        # ---- HAM pacer: ~3.5us of solid back-to-back MMs during the x-DMA
        # wait warms the PE clock gate before the DWT cascade begins.
        wup = psF.tile([P, P], F32, tag="psF", name="wup")
        for wi in range(32):
            nc.tensor.matmul(wup, idn_sb, idn_sb, start=(wi == 0), stop=(wi == 31))
        wud = pw.tile([P, 1], F32, name="wud")
        nc.vector.tensor_copy(out=wud, in_=wup[:, 0:1])

        # ---------------- LN1 with the full DWT cascade interleaved
        dwt_ctr = [0]

        def emit_dwt_group(lvl, g, last, src_main, src_bnd, low_dst, high_dst):
            pst = psA.tile([P, D], F32, tag="psA", name=f"dw{lvl}_{g}")
            nc.tensor.matmul(pst, fblk_sb[:, lvl * 3 + (2 if last else 0), :],
                             src_main, start=True, stop=last)
            if not last:
                nc.tensor.matmul(pst, fblk_sb[:2, lvl * 3 + 1, :], src_bnd,
                                 start=False, stop=True)
            Mg = 63 if last else 64
            lo = 64 * (g % 2)
            lt, li = low_dst
            ht, hi = high_dst
            nc.scalar.copy(out=lt[lo:lo + Mg, li, :], in_=pst[0:Mg, :])
            # the lead-in is DVE-bound (LN1 chain); keep only ~1/3 of the
            # high evacs on DVE, the rest on ACT, to balance the two queues
            dwt_ctr[0] += 1
            if dwt_ctr[0] % 3 == 0:
                nc.vector.tensor_copy(out=ht[lo:lo + Mg, hi, :], in_=pst[64:64 + Mg, :])
            else:
                nc.scalar.copy(out=ht[lo:lo + Mg, hi, :], in_=pst[64:64 + Mg, :])

        def emit_lvl1(g):
            emit_dwt_group(0, g, g == 31, xn_sb[:, g % 4, :],
                           None if g == 31 else xn_sb[:2, (g + 1) % 4, :],
                           (a1, g // 2), (d0, g // 2))

        def emit_lvl2(g):
            emit_dwt_group(1, g, g == 15, a1[:, g, :],
                           None if g == 15 else a1[:2, g + 1, :],
                           (a2, g // 2), (a1, g // 2))

        def emit_lvl3(g):
            emit_dwt_group(2, g, g == 7, a2[:, g, :],
                           None if g == 7 else a2[:2, g + 1, :],
                           (a3, g // 2), (a2, g // 2))

        def emit_ln1(i):
            st = pio.tile([P, 6], F32, tag="st", name=f"st{i}")
            nc.vector.bn_stats(out=st, in_=xr_sb[:, i, :])
            mv = pio.tile([P, 2], F32, tag="mv", name=f"mv{i}")
            nc.vector.bn_aggr(out=mv, in_=st)
            sd = pio.tile([P, 1], F32, tag="sd", name=f"sd{i}")
            nc.scalar.activation(out=sd, in_=mv[:, 1:2], func=AF.Sqrt, bias=eps_sb)
            rs = pio.tile([P, 1], F32, tag="rs", name=f"rs{i}")
            nc.vector.reciprocal(out=rs, in_=sd)
            nc.vector.tensor_scalar(
                out=xn_sb[:, i % 4, :], in0=xr_sb[:, i, :], scalar1=mv[:, 0:1],
                scalar2=rs, op0=OP.subtract, op1=OP.mult)

        # ------- chunk pipeline
        dsrc = [(d0, 0), (a1, 0), (a2, 0), (a3, 0)]
        combh = {}
        x2h = {}
        mvh = {}
        rsh = {}
        tmh = {}
        xn2h = {}

        def emit_interp_group(c, dt):
            if dt == 0:
                combh[c] = pcomb.tile([P, NDT, 512], BF, tag="comb", name=f"comb{c}")
            sch = plan["isched"][c]
            psi = psA.tile([P, 512], F32, tag="psA", name=f"pi{c}_{dt}")
            for q, (s, kt, K, idx) in enumerate(sch):
                dt_, db_ = dsrc[s]
                nc.tensor.matmul(
                    psi, dt_[:K, db_ + kt, P * dt:P * (dt + 1)], ubh[idx][:K, :],
                    start=(q == 0), stop=(q == len(sch) - 1))
            nc.vector.tensor_copy(out=combh[c][:, dt, :], in_=psi)

        def emit_proj(c):
            x2h[c] = []
            mvh[c] = []
            rsh[c] = []
            tmh[c] = []
            comb_c = combh.pop(c)
            for tj in range(4):
                ti = 4 * c + tj
                psp = psB.tile([P, D], F32, tag="psB", name=f"pp{ti}")
                for dt in range(NDT):
                    nc.tensor.matmul(
                        psp, comb_c[:, dt, P * tj:P * (tj + 1)], wg_sb[:, dt, :],
                        start=(dt == 0), stop=(dt == NDT - 1))
                x2t = px2.tile([P, D], F32, tag="x2t", name=f"x2t{ti}")
                nc.vector.tensor_add(out=x2t, in0=psp, in1=xr_sb[:, ti, :])
                x2h[c].append(x2t)
                st = pio.tile([P, 6], F32, tag="st", name=f"st2_{ti}")
                nc.vector.bn_stats(out=st, in_=x2t)
                mv = pio.tile([P, 2], F32, tag="mv", name=f"mv2_{ti}")
                nc.vector.bn_aggr(out=mv, in_=st)
                mvh[c].append(mv)
                sd = pio.tile([P, 1], F32, tag="sd", name=f"sd2_{ti}")
                nc.scalar.activation(out=sd, in_=mv[:, 1:2], func=AF.Sqrt, bias=eps_sb)
                rs = pio.tile([P, 1], F32, tag="rs", name=f"rs2_{ti}")
                nc.vector.reciprocal(out=rs, in_=sd)
                rsh[c].append(rs)
                tmt = ptm.tile([P, D], BF, tag="tmt", name=f"tmt{ti}")
                nc.vector.tensor_scalar(
                    out=tmt, in0=x2t, scalar1=mv[:, 0:1], scalar2=rs,
                    op0=OP.subtract, op1=OP.mult)
                tmh[c].append(tmt)

        def emit_transpose(c):
            xn2h[c] = pxn2.tile([P, NDT, 512], F8, tag="xn2f", name=f"xn2f{c}")
            tmts = tmh.pop(c)
            for dt in range(NDT):
                pstp = psB.tile([P, 512], BF, tag="psB", name=f"pt{c}_{dt}")
                for tj in range(4):
                    nc.tensor.transpose(
                        pstp[:, P * tj:P * (tj + 1)],
                        tmts[tj][:, P * dt:P * (dt + 1)], idn_sb)
                nc.scalar.copy(out=xn2h[c][:, dt, :], in_=pstp)
            mvh.pop(c)
            rsh.pop(c)

        def emit_ffn1_group(c, ft):
            hb = NFT * (c % 2)
            xn2f = xn2h[c]
            psh = psF.tile([P, 512], F32, tag="psF", name=f"ph{c}_{ft}")
            nc.tensor.matmul(psh, w1_sb[:, 0:2, P * ft:P * (ft + 1)],
                             xn2f[:, 0:2, :], start=True, stop=False, perf_mode=DR)
            nc.tensor.matmul(psh, w1_sb[:, 2:4, P * ft:P * (ft + 1)],
                             xn2f[:, 2:4, :], start=False, stop=True, perf_mode=DR)
            nc.scalar.activation(
                out=hdn[:, hb + ft, :], in_=psh, func=AF.Gelu,
                bias=b1c_sb[:, ft:ft + 1])

        def emit_ffn2(c, eng):
            hb = NFT * (c % 2)
            x2ts = x2h.pop(c)
            xn2h.pop(c)
            for tj in range(4):
                ti = 4 * c + tj
                pso = psO.tile([P, D], F32, tag="psO", name=f"po{ti}")
                for k in range(8):
                    nc.tensor.matmul(
                        pso, hdn[:, hb + 2 * k:hb + 2 * k + 2, P * tj:P * (tj + 1)],
                        w2_sb[:, 2 * k:2 * k + 2, :],
                        start=(k == 0), stop=False, perf_mode=DR)
                nc.tensor.matmul(pso, ones_sb[0:1, :], b2r_sb[:, :],
                                 start=False, stop=True)
                ot = pio.tile([P, D], F32, tag="ot", name=f"ot{ti}")
                nc.vector.tensor_add(out=ot, in0=pso, in1=x2ts[tj])
                eng.dma_start(out=out_d[P * ti:P * (ti + 1), :], in_=ot)

        def emit_prologue():
            for dt in range(NDT):
                emit_interp_group(0, dt)
            emit_proj(0)
            emit_transpose(0)

        def emit_bundle(c):
            if c + 2 < NCH:
                emit_ub_dma(c + 2, nc.gpsimd)
            for blk in range(4):
                emit_interp_group(c + 1, blk)
                for ft in range(4 * blk, 4 * blk + 4):
                    emit_ffn1_group(c, ft)
            emit_proj(c + 1)
            emit_ffn2(c, nc.gpsimd)
            emit_transpose(c + 1)

        def emit_last():
            # k-major FFN2 follows the gelu stream (no bulk stall)
            c = NCH - 1
            hb = NFT * (c % 2)
            psos = [psO.tile([P, D], F32, tag="psO", name=f"poL{j}") for j in range(2)] + \
                   [psA.tile([P, D], F32, tag="psA", name=f"poL{j}") for j in range(2, 4)]
            for q in range(16):
                emit_ffn1_group(c, q)
                if q % 2 == 1:
                    k = q // 2
                    for tj in range(4):
                        nc.tensor.matmul(
                            psos[tj], hdn[:, hb + 2 * k:hb + 2 * k + 2, P * tj:P * (tj + 1)],
                            w2_sb[:, 2 * k:2 * k + 2, :],
                            start=(k == 0), stop=False, perf_mode=DR)
            x2ts = x2h.pop(c)
            xn2h.pop(c)
            for tj in range(4):
                ti = 4 * c + tj
                nc.tensor.matmul(psos[tj], ones_sb[0:1, :], b2r_sb[:, :],
                                 start=False, stop=True)
                ot = pio.tile([P, D], F32, tag="ot", name=f"ot{ti}")
                nc.vector.tensor_add(out=ot, in0=psos[tj], in1=x2ts[tj])
                nc.gpsimd.dma_start(out=out_d[P * ti:P * (ti + 1), :], in_=ot)

        # cascade prefix each chunk's interp needs: d0 tile kt <- lvl1 groups
        # 2kt,2kt+1; d2 tile <- lvl2; d3/a3 tile <- lvl3.
        needs = []
        for c in range(NCH):
            mk = [0, 0, 0]
            for (s, kt, K, idx) in plan["isched"][c]:
                j = 0 if s == 0 else (1 if s == 1 else 2)
                mk[j] = max(mk[j], kt)
            needs.append((2 * mk[0] + 2, 2 * mk[1] + 2, 2 * mk[2] + 2))

        # ---- weave: LN1 steps + cascade groups + chunk bundles, each gated
        # on exact data prefixes, so PE goes dense (and HAM-warm) early.
        l1e = l2e = l3e = 0
        nextb = -1      # -1 = prologue pending, then bundle index

        def drain_casc():
            nonlocal l2e, l3e
            # last group of a level has no boundary read: one tile less needed
            while l2e < 16 and (2 * l2e + 3 if l2e < 15 else 32) <= l1e:
                emit_lvl2(l2e)
                l2e += 1
            while l3e < 8 and (2 * l3e + 3 if l3e < 7 else 16) <= l2e:
                emit_lvl3(l3e)
                l3e += 1

        def drain_bundles(cap):
            nonlocal nextb
            while nextb < cap:
                gate = needs[0] if nextb == -1 else (
                    needs[nextb + 1] if nextb < NCH - 1 else None)
                if gate is not None and not (
                        l1e >= gate[0] and l2e >= gate[1] and l3e >= gate[2]):
                    return
                if nextb == -1:
                    emit_prologue()
                elif nextb < NCH - 1:
                    emit_bundle(nextb)
                else:
                    return
                nextb += 1

        # weave only prologue + bundles 0..1 into LN1: enough to keep PE
        # dense (and HAM-warm) early, without burying late bundles' DVE
        # deps behind LN1 work (priority inversion, measured regression).
        for i in range(NT):
            emit_ln1(i)
            if i >= 1:
                emit_lvl1(i - 1)
                l1e = i
            drain_casc()
            drain_bundles(0)
        emit_lvl1(31)
        l1e = 32
        drain_casc()
        drain_bundles(NCH - 1)
        assert nextb == NCH - 1 and l2e == 16 and l3e == 8, (nextb, l2e, l3e)
        emit_last()
        ctx.close()
    nc.compile()
    return nc


_BUILT = {}


def _get_built():
    if "nc" not in _BUILT:
        plan = make_plan()
        _BUILT["plan"] = plan
        _BUILT["nc"] = build_nc(plan)
    return _BUILT["nc"], _BUILT["plan"]


def kernel(**inputs):
    from concourse.bass_utils import run_bass_kernel_spmd

    nc, plan = _get_built()
    consts = make_consts(inputs, plan)
    xr = make_xfold(inputs)
    in_maps = []
    for b in range(B):
        m = {"x": np.ascontiguousarray(xr[b])}
        m.update(consts)
        in_maps.append(m)
    res = run_bass_kernel_spmd(nc, in_maps, core_ids=list(range(B)))
    out = np.stack([res.results[b]["out"] for b in range(B)]).astype(np.float32)
    return out


# revision 24
# speedup vs baseline: 1.1011x; 1.1011x over previous
"""Trainium2 Bass kernel for nn_MultiHeadDaubechiesBlock (v2).

Data-parallel over batch B=8 across 8 NeuronCores (one sequence per core).
Per-core pipeline:
  LN1 (DVE bn_stats, bf16 x resident in SBUF) -> DWT as Toeplitz-block
  matmuls (token-major, lvl1 interleaved with LN1) -> merged linear-interp
  upsample matmuls (feature-major out) -> proj GEMM bf16 + residual
  (rank-1 bias terms folded into the uploaded x on the host) -> LN2 ->
  FFN1 + FFN2 as fp8e4 DoubleRow GEMMs (2x PE rate), exact gelu on ACT.
Software-pipelined chunk loop: FFN2(c) is deferred past interp/proj(c+1)
so the gelu stream always drains before FFN2 consumes it and PE never
stalls; no warmup pacer (HAM warms during the DWT burst).
"""
import numpy as np
import ml_dtypes

B, T, D, H, DH, LEVELS, FFN = 8, 4096, 512, 4, 128, 3, 2048
P = 128
NT = T // P          # 32 token tiles
NDT = D // P         # 4 feature tiles
NFT = FFN // P       # 16 ffn tiles
NCH = 8              # t-chunks of 512
NWS = [2047, 1023, 511]
LPADS = [4096, 2048, 1024]
EPS = 1e-5
BF16 = ml_dtypes.bfloat16
FP8 = ml_dtypes.float8_e4m3


# ----------------------------------------------------------------- plan
def _interp_mat(L, out_size=T):
    src = np.maximum((np.arange(out_size, dtype=np.float64) + 0.5) * (L / out_size) - 0.5, 0.0)
    i0 = np.clip(np.floor(src).astype(np.int64), 0, L - 1)
    i1 = np.minimum(i0 + 1, L - 1)
    w = src - i0
    U = np.zeros((out_size, L), np.float64)
    U[np.arange(out_size), i0] += 1.0 - w
    U[np.arange(out_size), i1] += w
    return U.astype(np.float32)


def make_plan():
    """Input-value-independent schedule + interp weight blocks."""
    dwt = []
    for lvl in range(LEVELS):
        nw = NWS[lvl]
        ng = (nw + 63) // 64
        dwt.append([(g, g == ng - 1) for g in range(ng)])

    Ls = [NWS[0], NWS[1], NWS[2], NWS[2]]
    Us = [_interp_mat(L) for L in Ls]
    ublks = []
    isched = [[] for _ in range(NCH)]
    for c in range(NCH):
        for s in range(4):
            U, L = Us[s], Ls[s]
            cols = U[512 * c:512 * (c + 1)]           # [512, L]
            nz = np.nonzero(cols.any(0))[0]
            for kt in range(nz.min() // P, nz.max() // P + 1):
                K = min(P, L - P * kt)
                blk = cols[:, P * kt:P * kt + K].T    # [K, 512]
                if not np.any(blk):
                    continue
                full = np.zeros((P, 512), np.float32)
                full[:K] = blk
                isched[c].append((s, kt, K, len(ublks)))
                ublks.append(full)
    ublk = np.stack(ublks)                            # [NB, 128, 512] f32
    return {"dwt": dwt, "isched": isched, "ublk": ublk, "nb": len(ublks)}


def _toeplitz(nw, Lp, f):
    F = np.zeros((nw, Lp), np.float32)
    for w in range(nw):
        F[w, 2 * w:2 * w + 4] = f
    return F


def make_consts(inputs, plan):
    """Host-side constants (depend on input values)."""
    h0, h1 = np.asarray(inputs["h0"]), np.asarray(inputs["h1"])
    f0 = h0[:, 0, :, 0].astype(np.float32)
    f1 = h1[:, 0, :, 0].astype(np.float32)
    ln1_g = np.asarray(inputs["ln1_g"], np.float32)
    ln2_g = np.asarray(inputs["ln2_g"], np.float32)
    ln2_b = np.asarray(inputs["ln2_b"], np.float32)
    proj_w = np.asarray(inputs["proj_w"], np.float32)
    w1 = np.asarray(inputs["w1"], np.float32)
    b1 = np.asarray(inputs["b1"], np.float32)
    w2 = np.asarray(inputs["w2"], np.float32)
    b2 = np.asarray(inputs["b2"], np.float32)

    # DWT lhsT blocks for 128-window groups, separate low/high PSUMs.
    # psum p (windows 128p..128p+127) = A @ srctile(2p) + B @ srctile(2p+1)
    # + C[:2] @ srctile(2p+2)[:2].  A: w<=63, B: w>=63, C: w=127 only.
    # Last psum of each level has 127 valid windows: use B with col 127
    # zeroed (Bl) and skip C.  Layout: [lvl*6 + ty*3 + {A,B,Bl}][128,128],
    # plus fbc [2, lvl*2+ty, 128] for the C strips.
    fblk = np.zeros((18, P, P), np.float32)
    fbc = np.zeros((2, 6, P), np.float32)
    for lvl in range(LEVELS):
        for ty, f in ((0, f0[lvl]), (1, f1[lvl])):
            A = fblk[lvl * 6 + ty * 3 + 0]
            Bm = fblk[lvl * 6 + ty * 3 + 1]
            for r in range(P):
                for w in range(P):
                    k = r - 2 * w
                    if 0 <= k < 4:
                        A[r, w] = f[k]
                    k = 128 + r - 2 * w
                    if 0 <= k < 4:
                        Bm[r, w] = f[k]
            Bl = fblk[lvl * 6 + ty * 3 + 2]
            Bl[:] = Bm
            Bl[:, 127] = 0.0
            for r in range(2):
                fbc[r, lvl * 2 + ty, 127] = f[r + 2]

    wg = (ln1_g[:, None] * proj_w)                # LN1 g fold
    w1g = (ln2_g[:, None] * w1)                   # LN2 g fold
    b1f = b1 + ln2_b @ w1                         # LN2 b fold

    return {
        "wg": wg.astype(BF16),
        "w1": w1g.astype(FP8),
        "w2": w2.astype(FP8),
        "fblk": fblk.reshape(18 * P, P).astype(BF16),         # [18*128, 128]
        "fbc": fbc.reshape(2, 6 * P).astype(BF16),            # [2, 6*128]
        "ublk": plan["ublk"].reshape(-1, 512).astype(BF16),   # [NB*128, 512]
        "b1c": np.ascontiguousarray(b1f.reshape(NFT, P).T.astype(np.float32)),  # [128,16]
        "b2r": b2.reshape(1, D).astype(BF16),                 # [1, 512]
        "idn": np.identity(P, np.float32).astype(BF16),       # [128,128]
    }


def make_xfold(inputs):
    """xr = x + 1*proj_b + m1*(ln1_b@proj_w), the rank-1 proj bias terms."""
    f0 = np.asarray(inputs["h0"])[:, 0, :, 0].astype(np.float32)
    f1 = np.asarray(inputs["h1"])[:, 0, :, 0].astype(np.float32)
    proj_b = np.asarray(inputs["proj_b"], np.float32)
    ln1_b = np.asarray(inputs["ln1_b"], np.float32)
    proj_w = np.asarray(inputs["proj_w"], np.float32)
    bW = ln1_b @ proj_w
    ones = np.ones((T, 1), np.float32)
    a = ones
    comb = np.zeros((T, 1), np.float32)
    Us = [_interp_mat(L) for L in [NWS[0], NWS[1], NWS[2], NWS[2]]]
    for lvl in range(LEVELS):
        ap_ = np.zeros((LPADS[lvl], 1), np.float32)
        ap_[:a.shape[0]] = a
        comb += Us[lvl] @ (_toeplitz(NWS[lvl], LPADS[lvl], f1[lvl]) @ ap_)
        a = _toeplitz(NWS[lvl], LPADS[lvl], f0[lvl]) @ ap_
    comb += Us[3] @ a
    m1 = comb[:, 0]                               # [T]
    x = np.asarray(inputs["x"], np.float32)
    xr = x + proj_b[None, None, :] + m1[None, :, None] * bW[None, None, :]
    return xr.astype(BF16)


# ----------------------------------------------------------------- bass
def build_nc(plan):
    import concourse.bacc as bacc
    import concourse.tile as tile
    from concourse import mybir

    F32, BF, F8 = mybir.dt.float32, mybir.dt.bfloat16, mybir.dt.float8e4
    AF = mybir.ActivationFunctionType
    OP = mybir.AluOpType
    DR = mybir.MatmulPerfMode.DoubleRow

    nc = bacc.Bacc("TRN2", target_bir_lowering=False, debug=False, name="daub2")
    x_d = nc.dram_tensor("x", [T, D], BF, kind="ExternalInput")
    out_d = nc.dram_tensor("out", [T, D], F32, kind="ExternalOutput")
    wg_d = nc.dram_tensor("wg", [D, D], BF, kind="ExternalInput")
    w1_d = nc.dram_tensor("w1", [D, FFN], F8, kind="ExternalInput")
    w2_d = nc.dram_tensor("w2", [FFN, D], F8, kind="ExternalInput")
    fblk_d = nc.dram_tensor("fblk", [18 * P, P], BF, kind="ExternalInput")
    fbc_d = nc.dram_tensor("fbc", [2, 6 * P], BF, kind="ExternalInput")
    ublk_d = nc.dram_tensor("ublk", [plan["nb"] * P, 512], BF, kind="ExternalInput")
    b1c_d = nc.dram_tensor("b1c", [P, NFT], F32, kind="ExternalInput")
    b2r_d = nc.dram_tensor("b2r", [1, D], BF, kind="ExternalInput")
    idn_d = nc.dram_tensor("idn", [P, P], BF, kind="ExternalInput")

    with tile.TileContext(nc) as tc:
        import contextlib
        ctx = contextlib.ExitStack()
        pw = ctx.enter_context(tc.tile_pool(name="pw", bufs=1))
        pbig = ctx.enter_context(tc.tile_pool(name="pbig", bufs=1))
        pio = ctx.enter_context(tc.tile_pool(name="pio", bufs=4))
        pu = ctx.enter_context(tc.tile_pool(name="pu", bufs=24))
        px2 = ctx.enter_context(tc.tile_pool(name="px2", bufs=8))
        pcomb = ctx.enter_context(tc.tile_pool(name="pcomb", bufs=2))
        pxn2 = ctx.enter_context(tc.tile_pool(name="pxn2", bufs=2))
        ptm = ctx.enter_context(tc.tile_pool(name="ptm", bufs=8))
        psA = ctx.enter_context(tc.tile_pool(name="psA", bufs=2, space="PSUM"))
        psB = ctx.enter_context(tc.tile_pool(name="psB", bufs=2, space="PSUM"))
        psF = ctx.enter_context(tc.tile_pool(name="psF", bufs=2, space="PSUM"))
        psO = ctx.enter_context(tc.tile_pool(name="psO", bufs=2, space="PSUM"))

        # ---- small consts
        idn_sb = pw.tile([P, P], BF, name="idn_sb")
        nc.sync.dma_start(out=idn_sb, in_=idn_d[:, :])
        fblk_sb = pw.tile([P, 18, P], BF, name="fblk_sb")
        nc.sync.dma_start(out=fblk_sb, in_=fblk_d.rearrange("(b p) m -> p b m", p=P))
        fbc_sb = pw.tile([2, 6, P], BF, name="fbc_sb")
        nc.sync.dma_start(out=fbc_sb, in_=fbc_d.rearrange("r (b m) -> r b m", m=P))
        b1c_sb = pw.tile([P, NFT], F32, name="b1c_sb")
        nc.sync.dma_start(out=b1c_sb, in_=b1c_d[:, :])
        b2r_sb = pw.tile([1, D], BF, name="b2r_sb")
        nc.sync.dma_start(out=b2r_sb, in_=b2r_d[:, :])
        eps_sb = pw.tile([P, 1], F32, name="eps_sb")
        nc.vector.memset(eps_sb, EPS)
        ones_sb = pw.tile([1, P], BF, name="ones_sb")
        nc.vector.memset(ones_sb, 1.0)

        # ---- big activations
        xr_sb = pbig.tile([P, NT, D], BF, name="xr_sb")      # resident x (folded)
        xn_sb = pbig.tile([P, 4, D], BF, name="xn_sb")       # LN1 out ring
        a1 = pbig.tile([P, 16, D], BF, name="a1")
        a2 = pbig.tile([P, 8, D], BF, name="a2")
        a3 = pbig.tile([P, 4, D], BF, name="a3")
        d0 = pbig.tile([P, 16, D], BF, name="d0")
        hdn = pbig.tile([P, NT, D], F8, name="hdn")
        wg_sb = pw.tile([P, NDT, D], BF, name="wg_sb")
        w1_sb = pw.tile([P, NDT, FFN], F8, name="w1_sb")
        w2_sb = pw.tile([P, NFT, D], F8, name="w2_sb")

        # zero the cascade-input pad rows (evacs later overwrite 96..126;
        # partition base must be 32-aligned so a 1-row memset is illegal)
        nc.vector.memset(a1[96:128, 15, :], 0.0)
        nc.vector.memset(a2[96:128, 7, :], 0.0)

        # ---- x upload (8 chunky DMAs), then weights, then ub c0/c1
        xrr = x_d.rearrange("(t p) d -> p t d", p=P)
        for j in range(8):
            nc.sync.dma_start(out=xr_sb[:, 4 * j:4 * j + 4, :], in_=xrr[:, 4 * j:4 * j + 4, :])
        nc.sync.dma_start(out=wg_sb, in_=wg_d.rearrange("(kt p) n -> p kt n", p=P))
        nc.sync.dma_start(out=w1_sb, in_=w1_d.rearrange("(kt p) n -> p kt n", p=P))
        nc.sync.dma_start(out=w2_sb, in_=w2_d.rearrange("(kt p) n -> p kt n", p=P))

        ubh = {}

        def emit_ub_dma(c, eng):
            for (s, kt, K, idx) in plan["isched"][c]:
                ut = pu.tile([P, 512], BF, tag="ub", name=f"ub{idx}")
                eng.dma_start(out=ut, in_=ublk_d[P * idx:P * (idx + 1), :])
                ubh[idx] = ut

        emit_ub_dma(0, nc.sync)
        emit_ub_dma(1, nc.sync)

        # ---- HAM pacer: ~3.5us of solid back-to-back MMs during the x-DMA
        # wait warms the PE clock gate before the DWT cascade begins.
        wup = psF.tile([P, P], F32, tag="psF", name="wup")
        for wi in range(32):
            nc.tensor.matmul(wup, idn_sb, idn_sb, start=(wi == 0), stop=(wi == 31))
        wud = pw.tile([P, 1], F32, name="wud")
        nc.vector.tensor_copy(out=wud, in_=wup[:, 0:1])

        # ---------------- LN1 with the full DWT cascade interleaved
        # 128-window psums: half the evac ops of 64-window merged groups;
        # all evacs live on ACT so the DVE queue only carries the LN1 chain.
        NPS = [16, 8, 4]

        def emit_dwt_psum(lvl, p, srcf, low_dst, high_dst):
            last = (p == NPS[lvl] - 1)
            Mg = 127 if last else 128
            for ty in range(2):
                dt_, di = (low_dst if ty == 0 else high_dst)
                pst = psA.tile([P, D], F32, tag="psA", name=f"dw{lvl}_{p}_{ty}")
                nc.tensor.matmul(pst, fblk_sb[:, lvl * 6 + ty * 3 + 0, :],
                                 srcf(2 * p), start=True, stop=False)
                nc.tensor.matmul(
                    pst, fblk_sb[:, lvl * 6 + ty * 3 + (2 if last else 1), :],
                    srcf(2 * p + 1), start=False, stop=last)
                if not last:
                    nc.tensor.matmul(pst, fbc_sb[:2, lvl * 2 + ty, :],
                                     srcf(2 * p + 2)[:2], start=False, stop=True)
                nc.scalar.copy(out=dt_[0:Mg, di, :], in_=pst[0:Mg, :])

        def emit_lvl1(p):
            emit_dwt_psum(0, p, lambda j: xn_sb[:, j % 4, :], (a1, p), (d0, p))

        def emit_lvl2(p):
            emit_dwt_psum(1, p, lambda j: a1[:, j, :], (a2, p), (a1, p))

        def emit_lvl3(p):
            emit_dwt_psum(2, p, lambda j: a2[:, j, :], (a3, p), (a2, p))

        def emit_ln1(i):
            st = pio.tile([P, 6], F32, tag="st", name=f"st{i}")
            nc.vector.bn_stats(out=st, in_=xr_sb[:, i, :])
            mv = pio.tile([P, 2], F32, tag="mv", name=f"mv{i}")
            nc.vector.bn_aggr(out=mv, in_=st)
            sd = pio.tile([P, 1], F32, tag="sd", name=f"sd{i}")
            nc.scalar.activation(out=sd, in_=mv[:, 1:2], func=AF.Sqrt, bias=eps_sb)
            rs = pio.tile([P, 1], F32, tag="rs", name=f"rs{i}")
            nc.vector.reciprocal(out=rs, in_=sd)
            nc.vector.tensor_scalar(
                out=xn_sb[:, i % 4, :], in0=xr_sb[:, i, :], scalar1=mv[:, 0:1],
                scalar2=rs, op0=OP.subtract, op1=OP.mult)

        # ------- chunk pipeline
        dsrc = [(d0, 0), (a1, 0), (a2, 0), (a3, 0)]
        combh = {}
        x2h = {}
        mvh = {}
        rsh = {}
        tmh = {}
        xn2h = {}

        def emit_interp_group(c, dt):
            if dt == 0:
                combh[c] = pcomb.tile([P, NDT, 512], BF, tag="comb", name=f"comb{c}")
            sch = plan["isched"][c]
            psi = psA.tile([P, 512], F32, tag="psA", name=f"pi{c}_{dt}")
            for q, (s, kt, K, idx) in enumerate(sch):
                dt_, db_ = dsrc[s]
                nc.tensor.matmul(
                    psi, dt_[:K, db_ + kt, P * dt:P * (dt + 1)], ubh[idx][:K, :],
                    start=(q == 0), stop=(q == len(sch) - 1))
            nc.vector.tensor_copy(out=combh[c][:, dt, :], in_=psi)

        def emit_proj(c):
            x2h[c] = []
            mvh[c] = []
            rsh[c] = []
            tmh[c] = []
            comb_c = combh.pop(c)
            for tj in range(4):
                ti = 4 * c + tj
                psp = psB.tile([P, D], F32, tag="psB", name=f"pp{ti}")
                for dt in range(NDT):
                    nc.tensor.matmul(
                        psp, comb_c[:, dt, P * tj:P * (tj + 1)], wg_sb[:, dt, :],
                        start=(dt == 0), stop=(dt == NDT - 1))
                x2t = px2.tile([P, D], F32, tag="x2t", name=f"x2t{ti}")
                nc.vector.tensor_add(out=x2t, in0=psp, in1=xr_sb[:, ti, :])
                x2h[c].append(x2t)
                st = pio.tile([P, 6], F32, tag="st", name=f"st2_{ti}")
                nc.vector.bn_stats(out=st, in_=x2t)
                mv = pio.tile([P, 2], F32, tag="mv", name=f"mv2_{ti}")
                nc.vector.bn_aggr(out=mv, in_=st)
                mvh[c].append(mv)
                sd = pio.tile([P, 1], F32, tag="sd", name=f"sd2_{ti}")
                nc.scalar.activation(out=sd, in_=mv[:, 1:2], func=AF.Sqrt, bias=eps_sb)
                rs = pio.tile([P, 1], F32, tag="rs", name=f"rs2_{ti}")
                nc.vector.reciprocal(out=rs, in_=sd)
                rsh[c].append(rs)
                tmt = ptm.tile([P, D], BF, tag="tmt", name=f"tmt{ti}")
                nc.vector.tensor_scalar(
                    out=tmt, in0=x2t, scalar1=mv[:, 0:1], scalar2=rs,
                    op0=OP.subtract, op1=OP.mult)
                tmh[c].append(tmt)

        def emit_transpose(c):
            xn2h[c] = pxn2.tile([P, NDT, 512], F8, tag="xn2f", name=f"xn2f{c}")
            tmts = tmh.pop(c)
            for dt in range(NDT):
                pstp = psB.tile([P, 512], BF, tag="psB", name=f"pt{c}_{dt}")
                for tj in range(4):
                    nc.tensor.transpose(
                        pstp[:, P * tj:P * (tj + 1)],
                        tmts[tj][:, P * dt:P * (dt + 1)], idn_sb)
                nc.scalar.copy(out=xn2h[c][:, dt, :], in_=pstp)
            mvh.pop(c)
            rsh.pop(c)

        def emit_ffn1_group(c, ft):
            hb = NFT * (c % 2)
            xn2f = xn2h[c]
            psh = psF.tile([P, 512], F32, tag="psF", name=f"ph{c}_{ft}")
            nc.tensor.matmul(psh, w1_sb[:, 0:2, P * ft:P * (ft + 1)],
                             xn2f[:, 0:2, :], start=True, stop=False, perf_mode=DR)
            nc.tensor.matmul(psh, w1_sb[:, 2:4, P * ft:P * (ft + 1)],
                             xn2f[:, 2:4, :], start=False, stop=True, perf_mode=DR)
            nc.scalar.activation(
                out=hdn[:, hb + ft, :], in_=psh, func=AF.Gelu,
                bias=b1c_sb[:, ft:ft + 1])

        def emit_ffn2(c, eng):
            hb = NFT * (c % 2)
            x2ts = x2h.pop(c)
            xn2h.pop(c)
            for tj in range(4):
                ti = 4 * c + tj
                pso = psO.tile([P, D], F32, tag="psO", name=f"po{ti}")
                for k in range(8):
                    nc.tensor.matmul(
                        pso, hdn[:, hb + 2 * k:hb + 2 * k + 2, P * tj:P * (tj + 1)],
                        w2_sb[:, 2 * k:2 * k + 2, :],
                        start=(k == 0), stop=False, perf_mode=DR)
                nc.tensor.matmul(pso, ones_sb[0:1, :], b2r_sb[:, :],
                                 start=False, stop=True)
                ot = pio.tile([P, D], F32, tag="ot", name=f"ot{ti}")
                nc.vector.tensor_add(out=ot, in0=pso, in1=x2ts[tj])
                eng.dma_start(out=out_d[P * ti:P * (ti + 1), :], in_=ot)

        def emit_prologue():
            for dt in range(NDT):
                emit_interp_group(0, dt)
            emit_proj(0)
            emit_transpose(0)

        def emit_bundle(c):
            if c + 2 < NCH:
                emit_ub_dma(c + 2, nc.gpsimd)
            for blk in range(4):
                emit_interp_group(c + 1, blk)
                for ft in range(4 * blk, 4 * blk + 4):
                    emit_ffn1_group(c, ft)
            emit_proj(c + 1)
            emit_ffn2(c, nc.gpsimd)
            emit_transpose(c + 1)

        def emit_last():
            # k-major FFN2 follows the gelu stream (no bulk stall)
            c = NCH - 1
            hb = NFT * (c % 2)
            psos = [psO.tile([P, D], F32, tag="psO", name=f"poL{j}") for j in range(2)] + \
                   [psA.tile([P, D], F32, tag="psA", name=f"poL{j}") for j in range(2, 4)]
            for q in range(16):
                emit_ffn1_group(c, q)
                if q % 2 == 1:
                    k = q // 2
                    for tj in range(4):
                        nc.tensor.matmul(
                            psos[tj], hdn[:, hb + 2 * k:hb + 2 * k + 2, P * tj:P * (tj + 1)],
                            w2_sb[:, 2 * k:2 * k + 2, :],
                            start=(k == 0), stop=False, perf_mode=DR)
            x2ts = x2h.pop(c)
            xn2h.pop(c)
            for tj in range(4):
                ti = 4 * c + tj
                nc.tensor.matmul(psos[tj], ones_sb[0:1, :], b2r_sb[:, :],
                                 start=False, stop=True)
                ot = pio.tile([P, D], F32, tag="ot", name=f"ot{ti}")
                nc.vector.tensor_add(out=ot, in0=psos[tj], in1=x2ts[tj])
                nc.gpsimd.dma_start(out=out_d[P * ti:P * (ti + 1), :], in_=ot)

        # cascade prefix each chunk's interp needs: d0/d2/d3/a3 tile kt is
        # written by the level's psum kt directly.
        needs = []
        for c in range(NCH):
            mk = [0, 0, 0]
            for (s, kt, K, idx) in plan["isched"][c]:
                j = 0 if s == 0 else (1 if s == 1 else 2)
                mk[j] = max(mk[j], kt)
            needs.append((mk[0] + 1, mk[1] + 1, mk[2] + 1))

        # ---- weave: LN1 steps + cascade groups + chunk bundles, each gated
        # on exact data prefixes, so PE goes dense (and HAM-warm) early.
        l1e = l2e = l3e = 0
        nextb = -1      # -1 = prologue pending, then bundle index

        def drain_casc():
            nonlocal l2e, l3e
            # psum p reads source tiles 2p..2p+2 (2p..2p+1 for the last)
            while l2e < 8 and (2 * l2e + 3 if l2e < 7 else 16) <= l1e:
                emit_lvl2(l2e)
                l2e += 1
            while l3e < 4 and (2 * l3e + 3 if l3e < 3 else 8) <= l2e:
                emit_lvl3(l3e)
                l3e += 1

        def drain_bundles(cap):
            nonlocal nextb
            while nextb < cap:
                gate = needs[0] if nextb == -1 else (
                    needs[nextb + 1] if nextb < NCH - 1 else None)
                if gate is not None and not (
                        l1e >= gate[0] and l2e >= gate[1] and l3e >= gate[2]):
                    return
                if nextb == -1:
                    emit_prologue()
                elif nextb < NCH - 1:
                    emit_bundle(nextb)
                else:
                    return
                nextb += 1

        # weave only prologue + bundles 0..1 into LN1: enough to keep PE
        # dense (and HAM-warm) early, without burying late bundles' DVE
        # deps behind LN1 work (priority inversion, measured regression).
        for i in range(NT):
            emit_ln1(i)
            # lvl1 psum p reads xn tiles 2p..2p+2 (30..31 for the last)
            while l1e < 16 and (2 * l1e + 2 if l1e < 15 else 31) <= i:
                emit_lvl1(l1e)
                l1e += 1
            drain_casc()
            drain_bundles(0)
        assert l1e == 16
        drain_casc()
        drain_bundles(NCH - 1)
        assert nextb == NCH - 1 and l2e == 8 and l3e == 4, (nextb, l2e, l3e)
        emit_last()
        ctx.close()
    nc.compile()
    return nc


_BUILT = {}


def _get_built():
    if "nc" not in _BUILT:
        plan = make_plan()
        _BUILT["plan"] = plan
        _BUILT["nc"] = build_nc(plan)
    return _BUILT["nc"], _BUILT["plan"]


def kernel(**inputs):
    from concourse.bass_utils import run_bass_kernel_spmd

    nc, plan = _get_built()
    consts = make_consts(inputs, plan)
    xr = make_xfold(inputs)
    in_maps = []
    for b in range(B):
        m = {"x": np.ascontiguousarray(xr[b])}
        m.update(consts)
        in_maps.append(m)
    res = run_bass_kernel_spmd(nc, in_maps, core_ids=list(range(B)))
    out = np.stack([res.results[b]["out"] for b in range(B)]).astype(np.float32)
    return out
